# revision 28
# baseline (speedup 1.0000x reference)
"""Trainium2 Bass kernel for nn_MultiHeadedAttentionWithGate.

Math (per molecule, validated against reference):
  The reference's reshapes are all flat views, so with u = "virtual row"
  (1024 per molecule), the computation is per-u over contiguous flat
  segments: K/V/M rows of 320 (10 nei x 32), X rows of 640 (10 x 64),
  q rows of 32.

Layout trick ("phase decomposition"): u = 4*g + r.  For fixed phase
r (0..3) and g on partitions, every tensor's u-row is a contiguous DRAM
segment (partition stride 2560 elems for X), and the projections
K/V/M[u-layout] decompose into matmuls over X^T chunks whose row sets
are stride-5 (rows 5g+d, d in 0..4) -- an affine AP.  The 20 (d, f-chunk)
X^T chunks per 128-g tile are the (f16) DMA-transposes of the 4 phases'
Xu tiles chunked by 128 columns.  All softmax/max/mean reductions are
then per-partition (free-axis) ops.  The neighbor-mean enters only via
a dot with Wg[64:128]; that dot is folded into the PE pass as 5 extra
N=1 matmuls per phase against the already-transposed X chunks.

Sharding: data-parallel over batch: 8 molecules per core x 8 cores.
"""

import sys

for _p in ("/opt/trn_rl_repo", "/root/.axon_site/_ro/trn_rl_repo"):
    if _p not in sys.path:
        sys.path.insert(0, _p)

from contextlib import ExitStack

import numpy as np

import concourse.bass as bass
import concourse.mybir as mybir
from concourse import bacc
from concourse.tile import TileContext

F16 = mybir.dt.float16
F32 = mybir.dt.float32
EXP = mybir.ActivationFunctionType.Exp
ADD = mybir.AluOpType.add
MAX = mybir.AluOpType.max
MULT = mybir.AluOpType.mult
AXL_X = mybir.AxisListType.X

N_CORES = 8
BM = 8          # molecules per core
A = 128         # atoms
NEI = 10
D = 256
D2 = 512


DEBUG = False


def build_nc(with_bias: bool, bg_val: float) -> bass.Bass:
    nc = bacc.Bacc("TRN2", target_bir_lowering=False)
    dbg = {}
    if DEBUG:
        for nm, shp in [("dbg_xt", [128, 128]), ("dbg_k", [128, 321]),
                        ("dbg_v", [128, 320]), ("dbg_m", [128, 320]),
                        ("dbg_score", [128, 10]), ("dbg_araw", [128, 4, 32]),
                        ("dbg_emax", [128, 4, 32]), ("dbg_gave", [128, 4]),
                        ("dbg_eg", [128, 4]), ("dbg_aden", [128, 4]),
                        ("dbg_qu", [128, 4, 32]), ("dbg_c2", [128, 4])]:
            dbg[nm] = nc.declare_dram_parameter(nm, shp, F32, isOutput=True)

    x_h = nc.declare_dram_parameter("x", [BM, A * NEI, D2], F32, isOutput=False)
    qin_h = nc.declare_dram_parameter("qin", [BM, A, D], F32, isOutput=False)
    wcat_h = nc.declare_dram_parameter("wcat", [128, 4, 768], F16, isOutput=False)
    wq_h = nc.declare_dram_parameter("wq", [128, 2, 256], F16, isOutput=False)
    ssel_h = nc.declare_dram_parameter("ssel", [128, 32], F16, isOutput=False)
    s2sel_h = nc.declare_dram_parameter("s2sel", [32, 128], F16, isOutput=False)
    wgc_h = nc.declare_dram_parameter("wg_cur", [128, 32], F32, isOutput=False)
    wge_h = nc.declare_dram_parameter("wg_emax", [128, 32], F32, isOutput=False)
    wgav_h = nc.declare_dram_parameter("wg_avc", [128, 5], F16, isOutput=False)
    if with_bias:
        bcat_h = nc.declare_dram_parameter("bcat", [1, 3, 256], F16, isOutput=False)
        bq_h = nc.declare_dram_parameter("bq", [1, 256], F16, isOutput=False)
        ones_h = nc.declare_dram_parameter("ones", [1, 128], F16, isOutput=False)
    out_h = nc.declare_dram_parameter("out", [BM, A, D], F32, isOutput=True)

    # flat per-molecule views: u = 4g + r = 512*G + 4*p + r
    x5 = (x_h[:].rearrange("b n c -> b (n c)")
          .rearrange("b (g p r t) -> b g r p t", g=2, p=128, r=4, t=640))
    q5 = (qin_h[:].rearrange("b a c -> b (a c)")
          .rearrange("b (g p r k) -> b g p r k", g=2, p=128, r=4, k=32))
    o5 = (out_h[:].rearrange("b a c -> b (a c)")
          .rearrange("b (g p r k) -> b g p r k", g=2, p=128, r=4, k=32))

    with TileContext(nc) as tc, ExitStack() as ctx:
        consts = ctx.enter_context(tc.tile_pool(name="consts", bufs=1))
        sb_x16 = ctx.enter_context(tc.tile_pool(name="x16", bufs=12))
        sb_xt = ctx.enter_context(tc.tile_pool(name="xt", bufs=44))
        sb_big = ctx.enter_context(tc.tile_pool(name="big", bufs=3))
        sb_ew = ctx.enter_context(tc.tile_pool(name="ew", bufs=4))
        sb_stash = ctx.enter_context(tc.tile_pool(name="stash", bufs=5))
        sb_q = ctx.enter_context(tc.tile_pool(name="qp", bufs=2))
        ps_proj = ctx.enter_context(tc.tile_pool(name="pp", bufs=2, space="PSUM"))
        ps_misc = ctx.enter_context(tc.tile_pool(name="pm", bufs=2, space="PSUM"))
        dram = ctx.enter_context(tc.tile_pool(name="dram", bufs=1, space="DRAM"))

        def cload(h, shape, dtype):
            t = consts.tile(shape, dtype, tag=h.name)
            nc.sync.dma_start(out=t, in_=h[:])
            return t

        wcat_t = cload(wcat_h, [128, 4, 768], F16)
        wq_t = cload(wq_h, [128, 2, 256], F16)
        ssel_t = cload(ssel_h, [128, 32], F16)
        s2sel_t = cload(s2sel_h, [32, 128], F16)
        wgc_t = cload(wgc_h, [128, 32], F32)
        wge_t = cload(wge_h, [128, 32], F32)
        wgav_t = cload(wgav_h, [128, 5], F16)
        if with_bias:
            bcat_t = cload(bcat_h, [1, 3, 256], F16)
            bq_t = cload(bq_h, [1, 256], F16)
            ones_t = cload(ones_h, [1, 128], F16)

        qdram = dram.tile([BM, A * D], F32)

        for mol in range(BM):
            # ---- q projection (natural layout) -> DRAM scratch ----
            qin16 = sb_q.tile([128, 256], F16, tag="qin16")
            nc.gpsimd.dma_start(out=qin16, in_=qin_h[mol])
            qT = []
            for w in range(2):
                t = sb_q.tile([128, 128], F16, tag="qT")
                nc.sync.dma_start(out=t, in_=qin16[:, 128 * w:128 * (w + 1)],
                                  transpose=True)
                qT.append(t)
            qpsum = ps_misc.tile([128, 256], F32, tag="pm")
            nc.tensor.matmul(qpsum, qT[0], wq_t[:, 0, :], start=True, stop=False)
            nc.tensor.matmul(qpsum, qT[1], wq_t[:, 1, :],
                             start=False, stop=not with_bias)
            if with_bias:
                nc.tensor.matmul(qpsum, ones_t, bq_t, start=False, stop=True)
            qnat = sb_q.tile([128, 256], F32, tag="qnat")
            nc.vector.tensor_copy(out=qnat, in_=qpsum)
            nc.sync.dma_start(
                out=qdram[mol].rearrange("(a c) -> a c", a=128), in_=qnat)
            q_read = qdram[mol].rearrange(
                "(g p r k) -> g p r k", g=2, p=128, r=4, k=32)

            for G in range(2):
                # ---- X load (cast f32->f16 in DMA) + DMA-transpose ----
                xu16 = []
                XT = {}
                for r in range(4):
                    t = sb_x16.tile([128, 640], F16, tag="x16")
                    nc.gpsimd.dma_start(out=t, in_=x5[mol, G, r])
                    xu16.append(t)
                for r in range(4):
                    for w in range(5):
                        d, fc = divmod(5 * r + w, 4)
                        t = sb_xt.tile([128, 128], F16, tag="xt")
                        nc.sync.dma_start(
                            out=t, in_=xu16[r][:, 128 * w:128 * (w + 1)],
                            transpose=True)
                        XT[(d, fc)] = t

                cur4 = sb_ew.tile([128, 4, 32], F32, tag="cur4")
                nc.sync.dma_start(out=cur4, in_=q5[mol, G])
                qu4 = sb_ew.tile([128, 4, 32], F32, tag="qu4")
                nc.sync.dma_start(out=qu4, in_=q_read[G])

                arawB = sb_stash.tile([128, 4, 32], F32, tag="arawB")
                emaxB = sb_ew.tile([128, 4, 32], F32, tag="emaxB")
                gaveB = sb_ew.tile([128, 4], F32, tag="gaveB")
                adenB = sb_ew.tile([128, 4], F32, tag="adenB")
                pg = ps_misc.tile([128, 4], F32, tag="pm", name="pg")

                for r in range(4):
                    # ---- projections K|V|M into one 3-bank psum tile ----
                    wA = 256 - 64 * r
                    ranges = [(r, 0, wA, 64 * r), (r + 1, wA, 320 - wA, 0)]
                    kvm = ps_proj.tile([128, 3, 320], F32, tag="pp",
                                       padded_shape=[128, 3, 512])
                    for (d, t0, wd, e0) in ranges:
                        for fc in range(4):
                            st = fc == 0
                            sp = (fc == 3) and not with_bias
                            for i in range(3):
                                nc.tensor.matmul(
                                    kvm[:, i, t0:t0 + wd], XT[(d, fc)],
                                    wcat_t[:, fc, 256 * i + e0:256 * i + e0 + wd],
                                    start=st, stop=sp)
                            # fold the neighbor-mean dot into the PE pass
                            w_ave = 4 * d + fc - 5 * r
                            if 0 <= w_ave < 5:
                                nc.tensor.matmul(
                                    pg[:, r:r + 1], XT[(d, fc)],
                                    wgav_t[:, w_ave:w_ave + 1],
                                    start=(w_ave == 0), stop=(w_ave == 4),
                                    skip_group_check=True)
                        if with_bias:
                            for i in range(3):
                                nc.tensor.matmul(
                                    kvm[:, i, t0:t0 + wd], ones_t,
                                    bcat_t[:, i, e0:e0 + wd],
                                    start=False, stop=True)

                    # ---- per-phase elementwise ----
                    smul = sb_big.tile([128, 320], F32, tag="smul")
                    nc.vector.tensor_mul(
                        smul, kvm[:, 0, 0:320],
                        qu4[:, r, :].unsqueeze(1).broadcast_to([128, 10, 32]))
                    score = sb_ew.tile([128, 10], F32, tag="score")
                    nc.vector.tensor_reduce(
                        out=score, in_=smul.rearrange("p (j k) -> p j k", j=10),
                        axis=AXL_X, op=ADD)
                    ex = sb_ew.tile([128, 10], F16, tag="ex")
                    nc.scalar.activation(out=ex, in_=score, func=EXP,
                                         accum_out=adenB[:, r:r + 1])
                    v16 = sb_big.tile([128, 320], F16, tag="v16")
                    nc.scalar.copy(out=v16, in_=kvm[:, 1, 0:320])
                    amul = sb_big.tile([128, 320], F16, tag="amul")
                    nc.gpsimd.tensor_mul(
                        amul, v16,
                        ex.unsqueeze(2).broadcast_to([128, 10, 32]))
                    nc.vector.tensor_reduce(
                        out=arawB[:, r, :],
                        in_=amul.rearrange("p (j k) -> p k j", j=10),
                        axis=AXL_X, op=ADD)
                    nc.vector.tensor_reduce(
                        out=emaxB[:, r, :],
                        in_=kvm[:, 2, 0:320].rearrange("p (j k) -> p k j", j=10),
                        axis=AXL_X, op=MAX)
                    if DEBUG and mol == 0 and G == 0 and r == 0:
                        kc = sb_big.tile([128, 320], F32, tag="dbgk")
                        nc.vector.tensor_copy(out=kc, in_=kvm[:, 0, :])
                        nc.sync.dma_start(out=dbg["dbg_k"][:, :320], in_=kc)
                        vc = sb_big.tile([128, 320], F32, tag="dbgv")
                        nc.vector.tensor_copy(out=vc, in_=kvm[:, 1, :320])
                        nc.sync.dma_start(out=dbg["dbg_v"][:], in_=vc)
                        mc = sb_big.tile([128, 320], F32, tag="dbgm")
                        nc.vector.tensor_copy(out=mc, in_=kvm[:, 2, :320])
                        nc.sync.dma_start(out=dbg["dbg_m"][:], in_=mc)
                        nc.sync.dma_start(out=dbg["dbg_score"][:], in_=score)
                        xtc = sb_big.tile([128, 128], F32, tag="dbgxt")
                        nc.vector.tensor_copy(out=xtc, in_=XT[(0, 0)])
                        nc.sync.dma_start(out=dbg["dbg_xt"][:], in_=xtc)

                # ---- gate logits (batched over the 4 phases) ----
                nc.vector.tensor_copy(out=gaveB, in_=pg)
                curp = sb_ew.tile([128, 4, 32], F32, tag="curp")
                nc.gpsimd.tensor_mul(
                    curp, cur4,
                    wgc_t.unsqueeze(1).broadcast_to([128, 4, 32]))
                gcurB = sb_ew.tile([128, 4], F32, tag="gcurB")
                nc.vector.tensor_reduce(out=gcurB, in_=curp, axis=AXL_X, op=ADD)
                emaxp = sb_ew.tile([128, 4, 32], F32, tag="emaxp")
                nc.gpsimd.tensor_mul(
                    emaxp, emaxB,
                    wge_t.unsqueeze(1).broadcast_to([128, 4, 32]))
                gemxB = sb_ew.tile([128, 4], F32, tag="gemxB")
                nc.vector.tensor_reduce(out=gemxB, in_=emaxp, axis=AXL_X, op=ADD)
                gl1 = sb_ew.tile([128, 4], F32, tag="gl1")
                nc.vector.tensor_add(gl1, gcurB, gemxB)
                gl2 = sb_ew.tile([128, 4], F32, tag="gl2")
                nc.vector.tensor_add(gl2, gl1, gaveB)
                egB = sb_stash.tile([128, 4], F32, tag="egB")
                nc.scalar.activation(out=egB, in_=gl2, func=EXP,
                                     bias=float(bg_val))
                egB16 = sb_stash.tile([128, 4], F16, tag="egB16")
                nc.vector.tensor_copy(out=egB16, in_=egB)
                raB = sb_stash.tile([128, 4], F32, tag="raB")
                nc.vector.reciprocal(out=raB, in_=adenB)

                if DEBUG and mol == 0 and G == 0:
                    nc.sync.dma_start(out=dbg["dbg_araw"][:], in_=arawB)
                    nc.sync.dma_start(out=dbg["dbg_emax"][:], in_=emaxB)
                    nc.sync.dma_start(out=dbg["dbg_gave"][:], in_=gaveB)
                    egc = sb_ew.tile([128, 4], F32, tag="dbgeg")
                    nc.vector.tensor_copy(out=egc, in_=egB)
                    nc.sync.dma_start(out=dbg["dbg_eg"][:], in_=egc)
                    nc.sync.dma_start(out=dbg["dbg_aden"][:], in_=adenB)
                    nc.sync.dma_start(out=dbg["dbg_qu"][:], in_=qu4)

                if G == 0:
                    st0 = (arawB, egB, egB16, raB)
                else:
                    c2B = {}
                    for gg in range(2):
                        c2B[gg] = sb_stash.tile([128, 4], F32, tag=f"c2B{gg}",
                                                name=f"c2B{gg}")
                    for r in range(4):
                        gd = ps_misc.tile([32, 1], F32, tag="pm")
                        nc.tensor.matmul(gd, ssel_t, st0[2][:, r:r + 1],
                                         start=True, stop=False)
                        nc.tensor.matmul(gd, ssel_t, egB16[:, r:r + 1],
                                         start=False, stop=True)
                        rg = sb_ew.tile([32, 1], F32, tag="rg")
                        nc.vector.reciprocal(out=rg, in_=gd)
                        rg16 = sb_ew.tile([32, 1], F16, tag="rg16")
                        nc.vector.tensor_copy(out=rg16, in_=rg)
                        inv = ps_misc.tile([128, 1], F32, tag="pm")
                        nc.tensor.matmul(inv, s2sel_t, rg16,
                                         start=True, stop=True)
                        for gg, (ar_g, eg_g, eg16_g, ra_g) in (
                                (0, st0), (1, (arawB, egB, egB16, raB))):
                            nc.vector.tensor_scalar(
                                out=c2B[gg][:, r:r + 1], in0=inv,
                                scalar1=ra_g[:, r:r + 1],
                                scalar2=eg_g[:, r:r + 1],
                                op0=MULT, op1=MULT)
                    if DEBUG and mol == 0:
                        nc.sync.dma_start(out=dbg["dbg_c2"][:], in_=c2B[0])
                    for gg, ar_g in ((0, st0[0]), (1, arawB)):
                        outB = sb_ew.tile([128, 4, 32], F32, tag="outB")
                        nc.gpsimd.tensor_mul(
                            outB, ar_g,
                            c2B[gg].unsqueeze(2).broadcast_to([128, 4, 32]))
                        nc.sync.dma_start(out=o5[mol, gg], in_=outB)
    nc.finalize()
    return nc


def _prep_consts(Wq, bq, Wk, bk, Wv, bv, Wam, bam, Wg, bg):
    wcat = np.empty((128, 4, 768), np.float16)
    for i, W in enumerate((Wk, Wv, Wam)):
        for fc in range(4):
            wcat[:, fc, 256 * i:256 * (i + 1)] = W[128 * fc:128 * (fc + 1), :]
    wq = np.empty((128, 2, 256), np.float16)
    for fc in range(2):
        wq[:, fc, :] = Wq[128 * fc:128 * (fc + 1), :]
    p = np.arange(128)
    ssel = (p[:, None] % 32 == np.arange(32)[None, :]).astype(np.float16)
    s2sel = ssel.T.copy()
    wg = np.asarray(Wg[:, 0], np.float32)
    # wg_avc[floc, w] = Wg[64 + ((128*w + floc) % 64)] / NEI
    wgav = np.empty((128, 5), np.float32)
    for w in range(5):
        wgav[:, w] = wg[64 + (np.arange(128) % 64)] / NEI
    consts = {
        "wcat": wcat, "wq": wq,
        "ssel": ssel, "s2sel": s2sel,
        "wg_cur": np.tile(wg[0:32], (128, 1)).astype(np.float32),
        "wg_emax": np.tile(wg[32:64], (128, 1)).astype(np.float32),
        "wg_avc": wgav.astype(np.float16),
    }
    with_bias = any(np.any(np.asarray(b) != 0) for b in (bq, bk, bv, bam))
    if with_bias:
        bcat = np.stack([np.asarray(bk), np.asarray(bv), np.asarray(bam)]
                        ).astype(np.float16)[None, :, :].reshape(1, 3, 256)
        consts["bcat"] = bcat
        consts["bq"] = np.asarray(bq, np.float16).reshape(1, 256)
        consts["ones"] = np.ones((1, 128), np.float16)
    return consts, with_bias, float(np.asarray(bg).reshape(-1)[0])


_CACHE = {}
TRACE = False       # set by test.py for profiling runs
LAST_RESULTS = None  # BassKernelResults from the most recent run


def kernel(input_multihead, input_q, Wq, bq, Wk, bk, Wv, bv, Wam, bam, Wg, bg):
    from concourse.bass_utils import run_bass_kernel_spmd

    consts, with_bias, bg_val = _prep_consts(
        Wq, bq, Wk, bk, Wv, bv, Wam, bam, Wg, bg)

    key = (with_bias, bg_val)
    if key not in _CACHE:
        _CACHE[key] = build_nc(with_bias, bg_val)
    nc = _CACHE[key]

    x = np.ascontiguousarray(np.asarray(input_multihead, np.float32))
    q = np.ascontiguousarray(np.asarray(input_q, np.float32))
    in_maps = []
    for c in range(N_CORES):
        m = {"x": x[BM * c:BM * (c + 1)], "qin": q[BM * c:BM * (c + 1)]}
        m.update(consts)
        in_maps.append(m)

    res = run_bass_kernel_spmd(nc, in_maps, list(range(N_CORES)), trace=TRACE)
    global LAST_RESULTS
    LAST_RESULTS = res
    return np.concatenate([res.results[c]["out"] for c in range(N_CORES)],
                          axis=0)


# revision 36
# speedup vs baseline: 2.5566x; 2.5566x over previous
"""Trainium2 Bass kernel for nn_MultiHeadedAttentionWithGate.

Math (per molecule, validated against reference):
  The reference's reshapes are all flat views, so with u = "virtual row"
  (1024 per molecule), the computation is per-u over contiguous flat
  segments: K/V/M rows of 320 (10 nei x 32), X rows of 640 (10 x 64),
  q rows of 32.

Layout trick ("phase decomposition"): u = 4*g + r.  For fixed phase
r (0..3) and g on partitions, every tensor's u-row is a contiguous DRAM
segment (partition stride 2560 elems for X), and the projections
K/V/M[u-layout] decompose into matmuls over X^T chunks whose row sets
are stride-5 (rows 5g+d, d in 0..4) -- an affine AP.  The 20 (d, f-chunk)
X^T chunks per 128-g tile are the (f16) DMA-transposes of the 4 phases'
Xu tiles chunked by 128 columns.  All softmax/max/mean reductions are
then per-partition (free-axis) ops.  The neighbor-mean enters only via
a dot with Wg[64:128]; that dot is folded into the PE pass as 5 extra
N=1 matmuls per phase against the already-transposed X chunks.

Sharding: data-parallel over batch: 8 molecules per core x 8 cores.
"""

import sys

for _p in ("/opt/trn_rl_repo", "/root/.axon_site/_ro/trn_rl_repo"):
    if _p not in sys.path:
        sys.path.insert(0, _p)

from contextlib import ExitStack

import numpy as np

import concourse.bass as bass
import concourse.mybir as mybir
from concourse import bacc
from concourse.tile import TileContext

F16 = mybir.dt.float16
F32 = mybir.dt.float32
EXP = mybir.ActivationFunctionType.Exp
ADD = mybir.AluOpType.add
MAX = mybir.AluOpType.max
MULT = mybir.AluOpType.mult
AXL_X = mybir.AxisListType.X

N_CORES = 8
BM = 8          # molecules per core
A = 128         # atoms
NEI = 10
D = 256
D2 = 512


DEBUG = False


def build_nc(with_bias: bool, bg_val: float) -> bass.Bass:
    nc = bacc.Bacc("TRN2", target_bir_lowering=False)
    dbg = {}
    if DEBUG:
        for nm, shp in [("dbg_xt", [128, 128]), ("dbg_k", [128, 321]),
                        ("dbg_v", [128, 320]), ("dbg_m", [128, 320]),
                        ("dbg_score", [128, 10]), ("dbg_araw", [128, 4, 32]),
                        ("dbg_emax", [128, 4, 32]), ("dbg_gave", [128, 4]),
                        ("dbg_eg", [128, 4]), ("dbg_aden", [128, 4]),
                        ("dbg_qu", [128, 4, 32]), ("dbg_c2", [128, 4])]:
            dbg[nm] = nc.declare_dram_parameter(nm, shp, F32, isOutput=True)

    x_h = nc.declare_dram_parameter("x", [BM, A * NEI, D2], F32, isOutput=False)
    qin_h = nc.declare_dram_parameter("qin", [BM, A, D], F32, isOutput=False)
    wcat_h = nc.declare_dram_parameter("wcat", [128, 4, 768], F16, isOutput=False)
    ident_h = nc.declare_dram_parameter("ident", [128, 128], F16, isOutput=False)
    wq_h = nc.declare_dram_parameter("wq", [128, 2, 256], F16, isOutput=False)
    ssel_h = nc.declare_dram_parameter("ssel", [128, 32], F16, isOutput=False)
    s2sel_h = nc.declare_dram_parameter("s2sel", [32, 128], F16, isOutput=False)
    wgc_h = nc.declare_dram_parameter("wg_cur", [128, 32], F32, isOutput=False)
    wge_h = nc.declare_dram_parameter("wg_emax", [128, 32], F32, isOutput=False)
    wgav_h = nc.declare_dram_parameter("wg_avc", [128, 5], F16, isOutput=False)
    if with_bias:
        bcat_h = nc.declare_dram_parameter("bcat", [1, 3, 256], F16, isOutput=False)
        bq_h = nc.declare_dram_parameter("bq", [1, 256], F16, isOutput=False)
        ones_h = nc.declare_dram_parameter("ones", [1, 128], F16, isOutput=False)
    out_h = nc.declare_dram_parameter("out", [BM, A, D], F32, isOutput=True)

    # flat per-molecule views: u = 4g + r = 512*G + 4*p + r
    x5 = (x_h[:].rearrange("b n c -> b (n c)")
          .rearrange("b (g p r t) -> b g r p t", g=2, p=128, r=4, t=640))
    q5 = (qin_h[:].rearrange("b a c -> b (a c)")
          .rearrange("b (g p r k) -> b g p r k", g=2, p=128, r=4, k=32))
    o5 = (out_h[:].rearrange("b a c -> b (a c)")
          .rearrange("b (g p r k) -> b g p r k", g=2, p=128, r=4, k=32))

    with TileContext(nc) as tc, ExitStack() as ctx:
        consts = ctx.enter_context(tc.tile_pool(name="consts", bufs=1))
        sb_x16 = ctx.enter_context(tc.tile_pool(name="x16", bufs=12))
        sb_xt = ctx.enter_context(tc.tile_pool(name="xt", bufs=44))
        sb_big = ctx.enter_context(tc.tile_pool(name="big", bufs=3))
        sb_ew = ctx.enter_context(tc.tile_pool(name="ew", bufs=4))
        sb_stash = ctx.enter_context(tc.tile_pool(name="stash", bufs=5))
        sb_q = ctx.enter_context(tc.tile_pool(name="qp", bufs=2))
        ps_proj = ctx.enter_context(tc.tile_pool(name="pp", bufs=2, space="PSUM"))
        ps_misc = ctx.enter_context(tc.tile_pool(name="pm", bufs=2, space="PSUM"))
        dram = ctx.enter_context(tc.tile_pool(name="dram", bufs=1, space="DRAM"))

        def cload(h, shape, dtype):
            t = consts.tile(shape, dtype, tag=h.name)
            nc.sync.dma_start(out=t, in_=h[:])
            return t

        wcat_t = cload(wcat_h, [128, 4, 768], F16)
        ident_t = cload(ident_h, [128, 128], F16)
        wq_t = cload(wq_h, [128, 2, 256], F16)
        ssel_t = cload(ssel_h, [128, 32], F16)
        s2sel_t = cload(s2sel_h, [32, 128], F16)
        wgc_t = cload(wgc_h, [128, 32], F32)
        wge_t = cload(wge_h, [128, 32], F32)
        wgav_t = cload(wgav_h, [128, 5], F16)
        if with_bias:
            bcat_t = cload(bcat_h, [1, 3, 256], F16)
            bq_t = cload(bq_h, [1, 256], F16)
            ones_t = cload(ones_h, [1, 128], F16)

        qdram = dram.tile([BM, A * D], F32)

        for mol in range(BM):
            # ---- q projection (natural layout) -> DRAM scratch ----
            qin16 = sb_q.tile([128, 256], F16, tag="qin16")
            nc.gpsimd.dma_start(out=qin16, in_=qin_h[mol])
            qtp = ps_misc.tile([128, 2, 128], F16, tag="pm", name="qtp")
            for w in range(2):
                nc.tensor.transpose(qtp[:, w, :],
                                    qin16[:, 128 * w:128 * (w + 1)], ident_t)
            qT = sb_q.tile([128, 2, 128], F16, tag="qT")
            nc.vector.tensor_copy(out=qT, in_=qtp)
            qpsum = ps_misc.tile([128, 256], F32, tag="pm")
            nc.tensor.matmul(qpsum, qT[:, 0, :], wq_t[:, 0, :],
                             start=True, stop=False)
            nc.tensor.matmul(qpsum, qT[:, 1, :], wq_t[:, 1, :],
                             start=False, stop=not with_bias)
            if with_bias:
                nc.tensor.matmul(qpsum, ones_t, bq_t, start=False, stop=True)
            qnat = sb_q.tile([128, 256], F32, tag="qnat")
            nc.vector.tensor_copy(out=qnat, in_=qpsum)
            nc.scalar.dma_start(
                out=qdram[mol].rearrange("(a c) -> a c", a=128), in_=qnat)
            q_read = qdram[mol].rearrange(
                "(g p r k) -> g p r k", g=2, p=128, r=4, k=32)

            for G in range(2):
                # ---- X load (cast f32->f16 in DMA) + PE transpose, with
                # 5 chunks batched per PSUM bank and one grouped copy ----
                xu16 = []
                XT = {}
                for r in range(4):
                    t = sb_x16.tile([128, 640], F16, tag="x16")
                    nc.gpsimd.dma_start(out=t, in_=x5[mol, G, r])
                    xu16.append(t)
                for r in range(4):
                    tp = ps_misc.tile([128, 5, 128], F16, tag="pm", name="tp")
                    for w in range(5):
                        nc.tensor.transpose(
                            tp[:, w, :], xu16[r][:, 128 * w:128 * (w + 1)],
                            ident_t)
                    xtb = sb_xt.tile([128, 5, 128], F16, tag="xt")
                    if r % 2 == 0:
                        nc.vector.tensor_copy(out=xtb, in_=tp)
                    else:
                        nc.scalar.copy(out=xtb, in_=tp)
                    for w in range(5):
                        d, fc = divmod(5 * r + w, 4)
                        XT[(d, fc)] = xtb[:, w, :]

                cur4 = sb_ew.tile([128, 4, 32], F32, tag="cur4")
                nc.sync.dma_start(out=cur4, in_=q5[mol, G])
                qu4 = sb_ew.tile([128, 4, 32], F32, tag="qu4")
                nc.sync.dma_start(out=qu4, in_=q_read[G])

                arawB = sb_stash.tile([128, 4, 32], F32, tag="arawB")
                emaxB = sb_ew.tile([128, 4, 32], F32, tag="emaxB")
                gaveB = sb_ew.tile([128, 4], F32, tag="gaveB")
                adenB = sb_ew.tile([128, 4], F32, tag="adenB")
                pg = ps_misc.tile([128, 4], F32, tag="pm", name="pg")

                for r in range(4):
                    # ---- projections K|V|M into one 3-bank psum tile ----
                    wA = 256 - 64 * r
                    ranges = [(r, 0, wA, 64 * r), (r + 1, wA, 320 - wA, 0)]
                    kvm = ps_proj.tile([128, 3, 320], F32, tag="pp",
                                       padded_shape=[128, 3, 512])
                    for (d, t0, wd, e0) in ranges:
                        for fc in range(4):
                            st = fc == 0
                            sp = (fc == 3) and not with_bias
                            for i in range(3):
                                nc.tensor.matmul(
                                    kvm[:, i, t0:t0 + wd], XT[(d, fc)],
                                    wcat_t[:, fc, 256 * i + e0:256 * i + e0 + wd],
                                    start=st, stop=sp)
                            # fold the neighbor-mean dot into the PE pass
                            w_ave = 4 * d + fc - 5 * r
                            if 0 <= w_ave < 5:
                                nc.tensor.matmul(
                                    pg[:, r:r + 1], XT[(d, fc)],
                                    wgav_t[:, w_ave:w_ave + 1],
                                    start=(w_ave == 0), stop=(w_ave == 4),
                                    skip_group_check=True)
                        if with_bias:
                            for i in range(3):
                                nc.tensor.matmul(
                                    kvm[:, i, t0:t0 + wd], ones_t,
                                    bcat_t[:, i, e0:e0 + wd],
                                    start=False, stop=True)

                    # ---- per-phase elementwise ----
                    smul = sb_big.tile([128, 320], F16, tag="smul")
                    nc.vector.tensor_mul(
                        smul, kvm[:, 0, 0:320],
                        qu4[:, r, :].unsqueeze(1).broadcast_to([128, 10, 32]))
                    score = sb_ew.tile([128, 10], F32, tag="score")
                    nc.vector.tensor_reduce(
                        out=score, in_=smul.rearrange("p (j k) -> p j k", j=10),
                        axis=AXL_X, op=ADD)
                    ex = sb_ew.tile([128, 10], F16, tag="ex")
                    nc.scalar.activation(out=ex, in_=score, func=EXP,
                                         accum_out=adenB[:, r:r + 1])
                    v16 = sb_big.tile([128, 320], F16, tag="v16")
                    nc.scalar.copy(out=v16, in_=kvm[:, 1, 0:320])
                    amul = sb_big.tile([128, 320], F16, tag="amul")
                    nc.gpsimd.tensor_mul(
                        amul, v16,
                        ex.unsqueeze(2).broadcast_to([128, 10, 32]))
                    nc.vector.tensor_reduce(
                        out=arawB[:, r, :],
                        in_=amul.rearrange("p (j k) -> p k j", j=10),
                        axis=AXL_X, op=ADD)
                    nc.vector.tensor_reduce(
                        out=emaxB[:, r, :],
                        in_=kvm[:, 2, 0:320].rearrange("p (j k) -> p k j", j=10),
                        axis=AXL_X, op=MAX)
                    if DEBUG and mol == 0 and G == 0 and r == 0:
                        kc = sb_big.tile([128, 320], F32, tag="dbgk")
                        nc.vector.tensor_copy(out=kc, in_=kvm[:, 0, :])
                        nc.sync.dma_start(out=dbg["dbg_k"][:, :320], in_=kc)
                        vc = sb_big.tile([128, 320], F32, tag="dbgv")
                        nc.vector.tensor_copy(out=vc, in_=kvm[:, 1, :320])
                        nc.sync.dma_start(out=dbg["dbg_v"][:], in_=vc)
                        mc = sb_big.tile([128, 320], F32, tag="dbgm")
                        nc.vector.tensor_copy(out=mc, in_=kvm[:, 2, :320])
                        nc.sync.dma_start(out=dbg["dbg_m"][:], in_=mc)
                        nc.sync.dma_start(out=dbg["dbg_score"][:], in_=score)
                        xtc = sb_big.tile([128, 128], F32, tag="dbgxt")
                        nc.vector.tensor_copy(out=xtc, in_=XT[(0, 0)])
                        nc.sync.dma_start(out=dbg["dbg_xt"][:], in_=xtc)

                # ---- gate logits (batched over the 4 phases) ----
                nc.vector.tensor_copy(out=gaveB, in_=pg)
                curp = sb_ew.tile([128, 4, 32], F32, tag="curp")
                nc.gpsimd.tensor_mul(
                    curp, cur4,
                    wgc_t.unsqueeze(1).broadcast_to([128, 4, 32]))
                gcurB = sb_ew.tile([128, 4], F32, tag="gcurB")
                nc.vector.tensor_reduce(out=gcurB, in_=curp, axis=AXL_X, op=ADD)
                emaxp = sb_ew.tile([128, 4, 32], F32, tag="emaxp")
                nc.gpsimd.tensor_mul(
                    emaxp, emaxB,
                    wge_t.unsqueeze(1).broadcast_to([128, 4, 32]))
                gemxB = sb_ew.tile([128, 4], F32, tag="gemxB")
                nc.vector.tensor_reduce(out=gemxB, in_=emaxp, axis=AXL_X, op=ADD)
                gl1 = sb_ew.tile([128, 4], F32, tag="gl1")
                nc.vector.tensor_add(gl1, gcurB, gemxB)
                gl2 = sb_ew.tile([128, 4], F32, tag="gl2")
                nc.vector.tensor_add(gl2, gl1, gaveB)
                egB = sb_stash.tile([128, 4], F32, tag="egB")
                nc.scalar.activation(out=egB, in_=gl2, func=EXP,
                                     bias=float(bg_val))
                egB16 = sb_stash.tile([128, 4], F16, tag="egB16")
                nc.vector.tensor_copy(out=egB16, in_=egB)
                raB = sb_stash.tile([128, 4], F32, tag="raB")
                nc.vector.reciprocal(out=raB, in_=adenB)

                if DEBUG and mol == 0 and G == 0:
                    nc.sync.dma_start(out=dbg["dbg_araw"][:], in_=arawB)
                    nc.sync.dma_start(out=dbg["dbg_emax"][:], in_=emaxB)
                    nc.sync.dma_start(out=dbg["dbg_gave"][:], in_=gaveB)
                    egc = sb_ew.tile([128, 4], F32, tag="dbgeg")
                    nc.vector.tensor_copy(out=egc, in_=egB)
                    nc.sync.dma_start(out=dbg["dbg_eg"][:], in_=egc)
                    nc.sync.dma_start(out=dbg["dbg_aden"][:], in_=adenB)
                    nc.sync.dma_start(out=dbg["dbg_qu"][:], in_=qu4)

                if G == 0:
                    st0 = (arawB, egB, egB16, raB)
                else:
                    c2B = {}
                    for gg in range(2):
                        c2B[gg] = sb_stash.tile([128, 4], F32, tag=f"c2B{gg}",
                                                name=f"c2B{gg}")
                    for r in range(4):
                        gd = ps_misc.tile([32, 1], F32, tag="pm")
                        nc.tensor.matmul(gd, ssel_t, st0[2][:, r:r + 1],
                                         start=True, stop=False)
                        nc.tensor.matmul(gd, ssel_t, egB16[:, r:r + 1],
                                         start=False, stop=True)
                        rg = sb_ew.tile([32, 1], F32, tag="rg")
                        nc.vector.reciprocal(out=rg, in_=gd)
                        rg16 = sb_ew.tile([32, 1], F16, tag="rg16")
                        nc.vector.tensor_copy(out=rg16, in_=rg)
                        inv = ps_misc.tile([128, 1], F32, tag="pm")
                        nc.tensor.matmul(inv, s2sel_t, rg16,
                                         start=True, stop=True)
                        for gg, (ar_g, eg_g, eg16_g, ra_g) in (
                                (0, st0), (1, (arawB, egB, egB16, raB))):
                            nc.vector.tensor_scalar(
                                out=c2B[gg][:, r:r + 1], in0=inv,
                                scalar1=ra_g[:, r:r + 1],
                                scalar2=eg_g[:, r:r + 1],
                                op0=MULT, op1=MULT)
                    if DEBUG and mol == 0:
                        nc.sync.dma_start(out=dbg["dbg_c2"][:], in_=c2B[0])
                    for gg, ar_g in ((0, st0[0]), (1, arawB)):
                        outB = sb_ew.tile([128, 4, 32], F32, tag="outB")
                        nc.gpsimd.tensor_mul(
                            outB, ar_g,
                            c2B[gg].unsqueeze(2).broadcast_to([128, 4, 32]))
                        nc.scalar.dma_start(out=o5[mol, gg], in_=outB)
    nc.finalize()
    return nc


def _prep_consts(Wq, bq, Wk, bk, Wv, bv, Wam, bam, Wg, bg):
    wcat = np.empty((128, 4, 768), np.float16)
    for i, W in enumerate((Wk, Wv, Wam)):
        for fc in range(4):
            wcat[:, fc, 256 * i:256 * (i + 1)] = W[128 * fc:128 * (fc + 1), :]
    wq = np.empty((128, 2, 256), np.float16)
    for fc in range(2):
        wq[:, fc, :] = Wq[128 * fc:128 * (fc + 1), :]
    p = np.arange(128)
    ssel = (p[:, None] % 32 == np.arange(32)[None, :]).astype(np.float16)
    s2sel = ssel.T.copy()
    wg = np.asarray(Wg[:, 0], np.float32)
    # wg_avc[floc, w] = Wg[64 + ((128*w + floc) % 64)] / NEI
    wgav = np.empty((128, 5), np.float32)
    for w in range(5):
        wgav[:, w] = wg[64 + (np.arange(128) % 64)] / NEI
    consts = {
        "wcat": wcat, "wq": wq,
        "ident": np.eye(128, dtype=np.float16),
        "ssel": ssel, "s2sel": s2sel,
        "wg_cur": np.tile(wg[0:32], (128, 1)).astype(np.float32),
        "wg_emax": np.tile(wg[32:64], (128, 1)).astype(np.float32),
        "wg_avc": wgav.astype(np.float16),
    }
    with_bias = any(np.any(np.asarray(b) != 0) for b in (bq, bk, bv, bam))
    if with_bias:
        bcat = np.stack([np.asarray(bk), np.asarray(bv), np.asarray(bam)]
                        ).astype(np.float16)[None, :, :].reshape(1, 3, 256)
        consts["bcat"] = bcat
        consts["bq"] = np.asarray(bq, np.float16).reshape(1, 256)
        consts["ones"] = np.ones((1, 128), np.float16)
    return consts, with_bias, float(np.asarray(bg).reshape(-1)[0])


_CACHE = {}
TRACE = False       # set by test.py for profiling runs
LAST_RESULTS = None  # BassKernelResults from the most recent run


def kernel(input_multihead, input_q, Wq, bq, Wk, bk, Wv, bv, Wam, bam, Wg, bg):
    from concourse.bass_utils import run_bass_kernel_spmd

    consts, with_bias, bg_val = _prep_consts(
        Wq, bq, Wk, bk, Wv, bv, Wam, bam, Wg, bg)

    key = (with_bias, bg_val)
    if key not in _CACHE:
        _CACHE[key] = build_nc(with_bias, bg_val)
    nc = _CACHE[key]

    x = np.ascontiguousarray(np.asarray(input_multihead, np.float32))
    q = np.ascontiguousarray(np.asarray(input_q, np.float32))
    in_maps = []
    for c in range(N_CORES):
        m = {"x": x[BM * c:BM * (c + 1)], "qin": q[BM * c:BM * (c + 1)]}
        m.update(consts)
        in_maps.append(m)

    res = run_bass_kernel_spmd(nc, in_maps, list(range(N_CORES)), trace=TRACE)
    global LAST_RESULTS
    LAST_RESULTS = res
    return np.concatenate([res.results[c]["out"] for c in range(N_CORES)],
                          axis=0)


# revision 43
# speedup vs baseline: 2.7748x; 1.0853x over previous
"""Trainium2 Bass kernel for nn_MultiHeadedAttentionWithGate.

Math (per molecule, validated against reference):
  The reference's reshapes are all flat views, so with u = "virtual row"
  (1024 per molecule), the computation is per-u over contiguous flat
  segments: K/V/M rows of 320 (10 nei x 32), X rows of 640 (10 x 64),
  q rows of 32.

Layout trick ("phase decomposition"): u = 4*g + r.  For fixed phase
r (0..3) and g on partitions, every tensor's u-row is a contiguous DRAM
segment (partition stride 2560 elems for X), and the projections
K/V/M[u-layout] decompose into matmuls over X^T chunks whose row sets
are stride-5 (rows 5g+d, d in 0..4) -- an affine AP.  The 20 (d, f-chunk)
X^T chunks per 128-g tile are the (f16) DMA-transposes of the 4 phases'
Xu tiles chunked by 128 columns.  All softmax/max/mean reductions are
then per-partition (free-axis) ops.  The neighbor-mean enters only via
a dot with Wg[64:128]; that dot is folded into the PE pass as 5 extra
N=1 matmuls per phase against the already-transposed X chunks.

Sharding: data-parallel over batch: 8 molecules per core x 8 cores.
"""

import sys

for _p in ("/opt/trn_rl_repo", "/root/.axon_site/_ro/trn_rl_repo"):
    if _p not in sys.path:
        sys.path.insert(0, _p)

from contextlib import ExitStack

import numpy as np

import concourse.bass as bass
import concourse.mybir as mybir
from concourse import bacc
from concourse.tile import TileContext

F16 = mybir.dt.float16
F32 = mybir.dt.float32
EXP = mybir.ActivationFunctionType.Exp
ADD = mybir.AluOpType.add
MAX = mybir.AluOpType.max
MULT = mybir.AluOpType.mult
AXL_X = mybir.AxisListType.X

N_CORES = 8
BM = 8          # molecules per core
A = 128         # atoms
NEI = 10
D = 256
D2 = 512


DEBUG = False


def build_nc(with_bias: bool, bg_val: float) -> bass.Bass:
    nc = bacc.Bacc("TRN2", target_bir_lowering=False)
    dbg = {}
    if DEBUG:
        for nm, shp in [("dbg_xt", [128, 128]), ("dbg_k", [128, 321]),
                        ("dbg_v", [128, 320]), ("dbg_m", [128, 320]),
                        ("dbg_score", [128, 10]), ("dbg_araw", [128, 4, 32]),
                        ("dbg_emax", [128, 4, 32]), ("dbg_gave", [128, 4]),
                        ("dbg_eg", [128, 4]), ("dbg_aden", [128, 4]),
                        ("dbg_qu", [128, 4, 32]), ("dbg_c2", [128, 4])]:
            dbg[nm] = nc.declare_dram_parameter(nm, shp, F32, isOutput=True)

    x_h = nc.declare_dram_parameter("x", [BM, A * NEI, D2], F32, isOutput=False)
    qin_h = nc.declare_dram_parameter("qin", [BM, A, D], F32, isOutput=False)
    wcat_h = nc.declare_dram_parameter("wcat", [128, 4, 768], F16, isOutput=False)
    ident_h = nc.declare_dram_parameter("ident", [128, 128], F16, isOutput=False)
    wq_h = nc.declare_dram_parameter("wq", [128, 2, 256], F16, isOutput=False)
    ssel_h = nc.declare_dram_parameter("ssel", [128, 32], F16, isOutput=False)
    s2sel_h = nc.declare_dram_parameter("s2sel", [32, 128], F16, isOutput=False)
    wgc_h = nc.declare_dram_parameter("wg_cur", [128, 32], F32, isOutput=False)
    wge_h = nc.declare_dram_parameter("wg_emax", [128, 32], F32, isOutput=False)
    wgav_h = nc.declare_dram_parameter("wg_avc", [128, 5], F16, isOutput=False)
    if with_bias:
        bcat_h = nc.declare_dram_parameter("bcat", [1, 3, 256], F16, isOutput=False)
        bq_h = nc.declare_dram_parameter("bq", [1, 256], F16, isOutput=False)
        ones_h = nc.declare_dram_parameter("ones", [1, 128], F16, isOutput=False)
    out_h = nc.declare_dram_parameter("out", [BM, A, D], F32, isOutput=True)

    # flat per-molecule views: u = 4g + r = 512*G + 4*p + r
    x5 = (x_h[:].rearrange("b n c -> b (n c)")
          .rearrange("b (g p r t) -> b g r p t", g=2, p=128, r=4, t=640))
    q5 = (qin_h[:].rearrange("b a c -> b (a c)")
          .rearrange("b (g p r k) -> b g p r k", g=2, p=128, r=4, k=32))
    o5 = (out_h[:].rearrange("b a c -> b (a c)")
          .rearrange("b (g p r k) -> b g p r k", g=2, p=128, r=4, k=32))

    with TileContext(nc) as tc, ExitStack() as ctx:
        consts = ctx.enter_context(tc.tile_pool(name="consts", bufs=1))
        sb_x16 = ctx.enter_context(tc.tile_pool(name="x16", bufs=12))
        sb_xt = ctx.enter_context(tc.tile_pool(name="xt", bufs=44))
        sb_big = ctx.enter_context(tc.tile_pool(name="big", bufs=3))
        sb_ew = ctx.enter_context(tc.tile_pool(name="ew", bufs=4))
        sb_stash = ctx.enter_context(tc.tile_pool(name="stash", bufs=5))
        sb_q = ctx.enter_context(tc.tile_pool(name="qp", bufs=2))
        ps_proj = ctx.enter_context(tc.tile_pool(name="pp", bufs=2, space="PSUM"))
        ps_tp = ctx.enter_context(tc.tile_pool(name="pt", bufs=1, space="PSUM"))
        ps_misc = ctx.enter_context(tc.tile_pool(name="pm", bufs=1, space="PSUM"))
        dram = ctx.enter_context(tc.tile_pool(name="dram", bufs=1, space="DRAM"))

        def cload(h, shape, dtype):
            t = consts.tile(shape, dtype, tag=h.name)
            nc.sync.dma_start(out=t, in_=h[:])
            return t

        wcat_t = cload(wcat_h, [128, 4, 768], F16)
        ident_t = cload(ident_h, [128, 128], F16)
        wq_t = cload(wq_h, [128, 2, 256], F16)
        ssel_t = cload(ssel_h, [128, 32], F16)
        s2sel_t = cload(s2sel_h, [32, 128], F16)
        wgc_t = cload(wgc_h, [128, 32], F32)
        wge_t = cload(wge_h, [128, 32], F32)
        wgav_t = cload(wgav_h, [128, 5], F16)
        if with_bias:
            bcat_t = cload(bcat_h, [1, 3, 256], F16)
            bq_t = cload(bq_h, [1, 256], F16)
            ones_t = cload(ones_h, [1, 128], F16)

        qdram = dram.tile([BM, A * D], F32)

        for mol in range(BM):
            # ---- q projection (natural layout) -> DRAM scratch ----
            qin16 = sb_q.tile([128, 256], F16, tag="qin16")
            nc.gpsimd.dma_start(out=qin16, in_=qin_h[mol])
            qtp = ps_tp.tile([128, 2, 128], F16, tag="pt", name="qtp")
            for w in range(2):
                nc.tensor.transpose(qtp[:, w, :],
                                    qin16[:, 128 * w:128 * (w + 1)], ident_t)
            qT = sb_q.tile([128, 2, 128], F16, tag="qT")
            nc.scalar.copy(out=qT, in_=qtp)
            qpsum = ps_misc.tile([128, 256], F32, tag="pm")
            nc.tensor.matmul(qpsum, qT[:, 0, :], wq_t[:, 0, :],
                             start=True, stop=False)
            nc.tensor.matmul(qpsum, qT[:, 1, :], wq_t[:, 1, :],
                             start=False, stop=not with_bias)
            if with_bias:
                nc.tensor.matmul(qpsum, ones_t, bq_t, start=False, stop=True)
            qnat = sb_q.tile([128, 256], F32, tag="qnat")
            nc.scalar.copy(out=qnat, in_=qpsum)
            nc.scalar.dma_start(
                out=qdram[mol].rearrange("(a c) -> a c", a=128), in_=qnat)
            q_read = qdram[mol].rearrange(
                "(g p r k) -> g p r k", g=2, p=128, r=4, k=32)

            for G in range(2):
                # ---- X load (cast f32->f16 in DMA) + PE transpose, with
                # 5 chunks batched per PSUM bank and one grouped copy ----
                xu16 = []
                XT = {}
                for r in range(4):
                    t = sb_x16.tile([128, 640], F16, tag="x16")
                    nc.gpsimd.dma_start(out=t, in_=x5[mol, G, r])
                    xu16.append(t)
                for r in range(4):
                    tp = ps_tp.tile([128, 5, 128], F16, tag="pt", name="tp")
                    for w in range(5):
                        nc.tensor.transpose(
                            tp[:, w, :], xu16[r][:, 128 * w:128 * (w + 1)],
                            ident_t)
                    xtb = sb_xt.tile([128, 5, 128], F16, tag="xt")
                    nc.scalar.copy(out=xtb, in_=tp)
                    for w in range(5):
                        d, fc = divmod(5 * r + w, 4)
                        XT[(d, fc)] = xtb[:, w, :]

                cur4 = sb_ew.tile([128, 4, 32], F32, tag="cur4")
                nc.sync.dma_start(out=cur4, in_=q5[mol, G])
                qu4 = sb_ew.tile([128, 4, 32], F32, tag="qu4")
                nc.sync.dma_start(out=qu4, in_=q_read[G])

                arawB = sb_stash.tile([128, 4, 32], F32, tag="arawB")
                emaxB = sb_ew.tile([128, 4, 32], F32, tag="emaxB")
                gaveB = sb_ew.tile([128, 4], F32, tag="gaveB")
                adenB = sb_ew.tile([128, 4], F32, tag="adenB")
                pg = ps_misc.tile([128, 4], F32, tag="pm", name="pg")

                for r in range(4):
                    # ---- projections K|V|M into one 3-bank psum tile ----
                    wA = 256 - 64 * r
                    ranges = [(r, 0, wA, 64 * r), (r + 1, wA, 320 - wA, 0)]
                    kvm = ps_proj.tile([128, 3, 320], F32, tag="pp",
                                       padded_shape=[128, 3, 512])
                    for (d, t0, wd, e0) in ranges:
                        for fc in range(4):
                            st = fc == 0
                            sp = (fc == 3) and not with_bias
                            for i in range(3):
                                nc.tensor.matmul(
                                    kvm[:, i, t0:t0 + wd], XT[(d, fc)],
                                    wcat_t[:, fc, 256 * i + e0:256 * i + e0 + wd],
                                    start=st, stop=sp)
                            # fold the neighbor-mean dot into the PE pass
                            w_ave = 4 * d + fc - 5 * r
                            if 0 <= w_ave < 5:
                                nc.tensor.matmul(
                                    pg[:, r:r + 1], XT[(d, fc)],
                                    wgav_t[:, w_ave:w_ave + 1],
                                    start=(w_ave == 0), stop=(w_ave == 4),
                                    skip_group_check=True)
                        if with_bias:
                            for i in range(3):
                                nc.tensor.matmul(
                                    kvm[:, i, t0:t0 + wd], ones_t,
                                    bcat_t[:, i, e0:e0 + wd],
                                    start=False, stop=True)

                    # ---- per-phase elementwise ----
                    smul = sb_big.tile([128, 320], F16, tag="smul")
                    nc.vector.tensor_mul(
                        smul, kvm[:, 0, 0:320],
                        qu4[:, r, :].unsqueeze(1).broadcast_to([128, 10, 32]))
                    score = sb_ew.tile([128, 10], F32, tag="score")
                    nc.vector.tensor_reduce(
                        out=score, in_=smul.rearrange("p (j k) -> p j k", j=10),
                        axis=AXL_X, op=ADD)
                    ex = sb_ew.tile([128, 10], F16, tag="ex")
                    nc.scalar.activation(out=ex, in_=score, func=EXP,
                                         accum_out=adenB[:, r:r + 1])
                    v16 = sb_big.tile([128, 320], F16, tag="v16")
                    nc.scalar.copy(out=v16, in_=kvm[:, 1, 0:320])
                    amul = sb_big.tile([128, 320], F16, tag="amul")
                    nc.gpsimd.tensor_mul(
                        amul, v16,
                        ex.unsqueeze(2).broadcast_to([128, 10, 32]))
                    nc.vector.tensor_reduce(
                        out=arawB[:, r, :],
                        in_=amul.rearrange("p (j k) -> p k j", j=10),
                        axis=AXL_X, op=ADD)
                    nc.vector.tensor_reduce(
                        out=emaxB[:, r, :],
                        in_=kvm[:, 2, 0:320].rearrange("p (j k) -> p k j", j=10),
                        axis=AXL_X, op=MAX)
                    if DEBUG and mol == 0 and G == 0 and r == 0:
                        kc = sb_big.tile([128, 320], F32, tag="dbgk")
                        nc.vector.tensor_copy(out=kc, in_=kvm[:, 0, :])
                        nc.sync.dma_start(out=dbg["dbg_k"][:, :320], in_=kc)
                        vc = sb_big.tile([128, 320], F32, tag="dbgv")
                        nc.vector.tensor_copy(out=vc, in_=kvm[:, 1, :320])
                        nc.sync.dma_start(out=dbg["dbg_v"][:], in_=vc)
                        mc = sb_big.tile([128, 320], F32, tag="dbgm")
                        nc.vector.tensor_copy(out=mc, in_=kvm[:, 2, :320])
                        nc.sync.dma_start(out=dbg["dbg_m"][:], in_=mc)
                        nc.sync.dma_start(out=dbg["dbg_score"][:], in_=score)
                        xtc = sb_big.tile([128, 128], F32, tag="dbgxt")
                        nc.vector.tensor_copy(out=xtc, in_=XT[(0, 0)])
                        nc.sync.dma_start(out=dbg["dbg_xt"][:], in_=xtc)

                # ---- gate logits (batched over the 4 phases) ----
                nc.vector.tensor_copy(out=gaveB, in_=pg)
                curp = sb_ew.tile([128, 4, 32], F32, tag="curp")
                nc.gpsimd.tensor_mul(
                    curp, cur4,
                    wgc_t.unsqueeze(1).broadcast_to([128, 4, 32]))
                gcurB = sb_ew.tile([128, 4], F32, tag="gcurB")
                nc.vector.tensor_reduce(out=gcurB, in_=curp, axis=AXL_X, op=ADD)
                emaxp = sb_ew.tile([128, 4, 32], F32, tag="emaxp")
                nc.gpsimd.tensor_mul(
                    emaxp, emaxB,
                    wge_t.unsqueeze(1).broadcast_to([128, 4, 32]))
                gemxB = sb_ew.tile([128, 4], F32, tag="gemxB")
                nc.vector.tensor_reduce(out=gemxB, in_=emaxp, axis=AXL_X, op=ADD)
                gl1 = sb_ew.tile([128, 4], F32, tag="gl1")
                nc.vector.tensor_add(gl1, gcurB, gemxB)
                gl2 = sb_ew.tile([128, 4], F32, tag="gl2")
                nc.vector.tensor_add(gl2, gl1, gaveB)
                egB = sb_stash.tile([128, 4], F32, tag="egB")
                nc.scalar.activation(out=egB, in_=gl2, func=EXP,
                                     bias=float(bg_val))
                egB16 = sb_stash.tile([128, 4], F16, tag="egB16")
                nc.vector.tensor_copy(out=egB16, in_=egB)
                raB = sb_stash.tile([128, 4], F32, tag="raB")
                nc.vector.reciprocal(out=raB, in_=adenB)

                if DEBUG and mol == 0 and G == 0:
                    nc.sync.dma_start(out=dbg["dbg_araw"][:], in_=arawB)
                    nc.sync.dma_start(out=dbg["dbg_emax"][:], in_=emaxB)
                    nc.sync.dma_start(out=dbg["dbg_gave"][:], in_=gaveB)
                    egc = sb_ew.tile([128, 4], F32, tag="dbgeg")
                    nc.vector.tensor_copy(out=egc, in_=egB)
                    nc.sync.dma_start(out=dbg["dbg_eg"][:], in_=egc)
                    nc.sync.dma_start(out=dbg["dbg_aden"][:], in_=adenB)
                    nc.sync.dma_start(out=dbg["dbg_qu"][:], in_=qu4)

                if G == 0:
                    st0 = (arawB, egB, egB16, raB)
                else:
                    gd = ps_misc.tile([32, 4], F32, tag="pm", name="gd")
                    for r in range(4):
                        nc.tensor.matmul(gd[:, r:r + 1], ssel_t,
                                         st0[2][:, r:r + 1],
                                         start=True, stop=False)
                        nc.tensor.matmul(gd[:, r:r + 1], ssel_t,
                                         egB16[:, r:r + 1],
                                         start=False, stop=True)
                    rg = sb_ew.tile([32, 4], F32, tag="rg")
                    nc.vector.reciprocal(out=rg, in_=gd)
                    rg16 = sb_ew.tile([32, 4], F16, tag="rg16")
                    nc.vector.tensor_copy(out=rg16, in_=rg)
                    inv = ps_misc.tile([128, 4], F32, tag="pm", name="inv")
                    for r in range(4):
                        nc.tensor.matmul(inv[:, r:r + 1], s2sel_t,
                                         rg16[:, r:r + 1],
                                         start=True, stop=True)
                    c2B = {}
                    for gg, (ar_g, eg_g, eg16_g, ra_g) in (
                            (0, st0), (1, (arawB, egB, egB16, raB))):
                        t1 = sb_ew.tile([128, 4], F32, tag="t1", name="t1")
                        nc.vector.tensor_mul(t1, inv, ra_g)
                        c2B[gg] = sb_stash.tile([128, 4], F32, tag=f"c2B{gg}",
                                                name=f"c2B{gg}")
                        nc.vector.tensor_mul(c2B[gg], t1, eg_g)
                    if DEBUG and mol == 0:
                        nc.sync.dma_start(out=dbg["dbg_c2"][:], in_=c2B[0])
                    for gg, ar_g in ((0, st0[0]), (1, arawB)):
                        outB = sb_ew.tile([128, 4, 32], F32, tag="outB")
                        nc.gpsimd.tensor_mul(
                            outB, ar_g,
                            c2B[gg].unsqueeze(2).broadcast_to([128, 4, 32]))
                        nc.scalar.dma_start(out=o5[mol, gg], in_=outB)
    nc.finalize()
    return nc


def _prep_consts(Wq, bq, Wk, bk, Wv, bv, Wam, bam, Wg, bg):
    wcat = np.empty((128, 4, 768), np.float16)
    for i, W in enumerate((Wk, Wv, Wam)):
        for fc in range(4):
            wcat[:, fc, 256 * i:256 * (i + 1)] = W[128 * fc:128 * (fc + 1), :]
    wq = np.empty((128, 2, 256), np.float16)
    for fc in range(2):
        wq[:, fc, :] = Wq[128 * fc:128 * (fc + 1), :]
    p = np.arange(128)
    ssel = (p[:, None] % 32 == np.arange(32)[None, :]).astype(np.float16)
    s2sel = ssel.T.copy()
    wg = np.asarray(Wg[:, 0], np.float32)
    # wg_avc[floc, w] = Wg[64 + ((128*w + floc) % 64)] / NEI
    wgav = np.empty((128, 5), np.float32)
    for w in range(5):
        wgav[:, w] = wg[64 + (np.arange(128) % 64)] / NEI
    consts = {
        "wcat": wcat, "wq": wq,
        "ident": np.eye(128, dtype=np.float16),
        "ssel": ssel, "s2sel": s2sel,
        "wg_cur": np.tile(wg[0:32], (128, 1)).astype(np.float32),
        "wg_emax": np.tile(wg[32:64], (128, 1)).astype(np.float32),
        "wg_avc": wgav.astype(np.float16),
    }
    with_bias = any(np.any(np.asarray(b) != 0) for b in (bq, bk, bv, bam))
    if with_bias:
        bcat = np.stack([np.asarray(bk), np.asarray(bv), np.asarray(bam)]
                        ).astype(np.float16)[None, :, :].reshape(1, 3, 256)
        consts["bcat"] = bcat
        consts["bq"] = np.asarray(bq, np.float16).reshape(1, 256)
        consts["ones"] = np.ones((1, 128), np.float16)
    return consts, with_bias, float(np.asarray(bg).reshape(-1)[0])


_CACHE = {}
TRACE = False       # set by test.py for profiling runs
LAST_RESULTS = None  # BassKernelResults from the most recent run


def kernel(input_multihead, input_q, Wq, bq, Wk, bk, Wv, bv, Wam, bam, Wg, bg):
    from concourse.bass_utils import run_bass_kernel_spmd

    consts, with_bias, bg_val = _prep_consts(
        Wq, bq, Wk, bk, Wv, bv, Wam, bam, Wg, bg)

    key = (with_bias, bg_val)
    if key not in _CACHE:
        _CACHE[key] = build_nc(with_bias, bg_val)
    nc = _CACHE[key]

    x = np.ascontiguousarray(np.asarray(input_multihead, np.float32))
    q = np.ascontiguousarray(np.asarray(input_q, np.float32))
    in_maps = []
    for c in range(N_CORES):
        m = {"x": x[BM * c:BM * (c + 1)], "qin": q[BM * c:BM * (c + 1)]}
        m.update(consts)
        in_maps.append(m)

    res = run_bass_kernel_spmd(nc, in_maps, list(range(N_CORES)), trace=TRACE)
    global LAST_RESULTS
    LAST_RESULTS = res
    return np.concatenate([res.results[c]["out"] for c in range(N_CORES)],
                          axis=0)
